# revision 33
# baseline (speedup 1.0000x reference)
"""Trainium2 Bass kernel for windowed mean-pooling (segment_reduce).

Computes, for each (batch b, window w):
    out[b, w, :] = mean over t in [begins[b,w], ends'[b,w]) of features[b, t, :]
where ends' = clip(ends, begins, begins + 8) (the reference gathers at most
MAX_WINDOW=8 tokens) and empty windows produce 0 (count clamped to >= 1).

Strategy (data-parallel over batch, one sample per NeuronCore):
  - The kernel is HBM-bound, so input bytes are minimized: features ship as
    fp16 (6.3 MB instead of 12.6 MB fp32; ~2e-4 rel err on the windowed
    means, and fp16 matmuls run at full PE rate unlike fp32 which lowers
    to two HW passes), begins/ends metadata as int16 broadcast rows (1 MB).
  - Slab layout in SBUF: token t on partition (t % 128), K-tile (t // 128).
  - For each 128-window output block: out_block = S^T @ F on the
    TensorEngine, where S[t, w] = (begins[w] <= t < ends[w]) is built per
    K-tile by the VectorEngine directly from the int16 rows with fused
    compare ops (S in fp16: 0/1 exact). Accumulate over the block's
    K-tiles in PSUM, scale rows by 1/count on the ScalarEngine
    (activation Copy with per-partition scale), DMA out.
  - Per-block K-tile ranges come from the host (actual index data), taking
    the union across the 8 cores so one SPMD program serves all cores
    (masks are zero outside a core's true range -> contributes nothing).
  - DMA assignment: features via GPSIMD SWDGE (descriptor generation off
    the critical sequencers, small chunks first so the PE starts early),
    metadata on SP, outputs on ACT.
"""

import os
import sys

import numpy as np

for _p in ("/opt/trn_rl_repo", "/root/.axon_site/_ro/trn_rl_repo"):
    if os.path.isdir(_p) and _p not in sys.path:
        sys.path.insert(0, _p)

from concourse import bacc, mybir  # noqa: E402
import concourse.tile as tile  # noqa: E402
from concourse.bass_utils import run_bass_kernel_spmd  # noqa: E402

B, T, D, W = 8, 4096, 768, 2048
MAXWIN = 8
P = 128
NBLK = W // P  # 16 window blocks of 128 windows
NKT = T // P  # 32 K-tiles of 128 tokens
FCHUNKS = (1, 1, 2, 4, 4, 4, 4, 4, 4, 2, 1, 1)  # K-tiles per feature DMA chunk
MCH = 512  # windows per metadata DMA chunk
F32 = mybir.dt.float32
FP16 = mybir.dt.float16
I16 = mybir.dt.int16


def _build_program(klo, khi):
    """Build the SPMD Bass program given per-block K-tile ranges [klo, khi)."""
    nc = bacc.Bacc(None)

    fhi_d = nc.declare_dram_parameter("fhi", [P, NKT, D], FP16, isOutput=False)
    meta = nc.declare_dram_parameter("meta", [W // MCH, P, 2, MCH], I16, isOutput=False)
    ioiv = nc.declare_dram_parameter("ioiv", [P, P], F32, isOutput=False)
    out_d = nc.declare_dram_parameter("out", [W, D], F32, isOutput=True)

    # token t = n*128 + p -> fhi[p, n, d] (host-shuffled for contiguous
    # per-partition DMA descriptors); window w = i*128 + p -> [p, i, d]
    fhi_r = fhi_d[:]
    out_r = out_d[:].rearrange("(n p) d -> p n d", p=P)

    # For each K-tile, the contiguous span of blocks that consume it.
    strip_rng = {}
    for k in range(NKT):
        blks = [i for i in range(NBLK) if klo[i] <= k < khi[i]]
        if blks:
            strip_rng[k] = (min(blks), max(blks) + 1)

    with tile.TileContext(nc) as tc:
        with (
            tc.tile_pool(name="metap", bufs=1) as meta_pool,
            tc.tile_pool(name="fslab", bufs=1) as f_pool,
            tc.tile_pool(name="m2p", bufs=3) as m2_pool,
            tc.tile_pool(name="maskp", bufs=6) as mask_pool,
            tc.tile_pool(name="outp", bufs=4) as out_pool,
            tc.tile_pool(name="psum", bufs=4, space="PSUM") as psum_pool,
        ):
            # iota [P, :NKT] (iota[p, k] = 128k + p), 1/count [P, NKT:NKT+NBLK],
            # zero-padded to [P, 128] so DMA descriptors stay >= 512 B.
            ioiv_sb = meta_pool.tile([P, P], F32)
            nc.sync.dma_start(out=ioiv_sb[:], in_=ioiv[:])
            io_sb = ioiv_sb[:, 0:NKT]
            iv_sb = ioiv_sb[:, NKT : NKT + NBLK]

            # begins/ends rows (pre-broadcast by the host, int16), one tile
            # per 512-window chunk so early strips only wait for their chunk.
            be_tiles = []
            for c in range(W // MCH):
                sl = slice(c * MCH, (c + 1) * MCH)
                bt = meta_pool.tile([P, 2, MCH], I16, name=f"be{c}", tag=f"be{c}")
                nc.sync.dma_start(out=bt[:], in_=meta[:][c])
                be_tiles.append(bt)

            # Feature slab chunks (fp16), small chunks first.
            fhi_tiles = []
            k2chunk = []
            k0 = 0
            for j, sz in enumerate(FCHUNKS):
                fh = f_pool.tile([P, sz, D], FP16, name=f"fh{j}", tag=f"fh{j}")
                nc.gpsimd.dma_start(out=fh[:], in_=fhi_r[:, k0 : k0 + sz, :])
                fhi_tiles.append(fh)
                for s in range(sz):
                    k2chunk.append((j, s))
                k0 += sz
            assert k0 == NKT

            # Per-K-tile mask strips over the span of blocks that use them,
            # in [token, window] layout: mask[p, w] = (b[w] <= t) * (e[w] > t)
            # with t = 128k + p.
            masks = {}
            for k in sorted(strip_rng):
                blo, bhi = strip_rng[k]
                wlo, whi = blo * P, bhi * P
                wn = whi - wlo
                m2 = m2_pool.tile([P, wn], FP16, name=f"m2_{k}", tag="m2")
                msk = mask_pool.tile([P, wn], FP16, name=f"mask_{k}", tag="mask")
                w0 = wlo
                while w0 < whi:  # split at metadata-chunk boundaries
                    w1 = min(whi, (w0 // MCH + 1) * MCH)
                    bt = be_tiles[w0 // MCH]
                    osl = slice(w0 - wlo, w1 - wlo)
                    csl = slice(w0 % MCH, w0 % MCH + (w1 - w0))
                    nc.vector.tensor_scalar(
                        m2[:, osl], bt[:, 1, csl], io_sb[:, k : k + 1], None,
                        mybir.AluOpType.is_gt,
                    )
                    nc.vector.scalar_tensor_tensor(
                        msk[:, osl], bt[:, 0, csl], io_sb[:, k : k + 1], m2[:, osl],
                        mybir.AluOpType.is_le, mybir.AluOpType.mult,
                    )
                    w0 = w1
                masks[k] = (msk, blo)

            for i in range(NBLK):
                ps = psum_pool.tile([P, D], F32, name=f"ps{i}", tag="ps")
                for k in range(klo[i], khi[i]):
                    msk, blo = masks[k]
                    lh = msk[:, (i - blo) * P : (i - blo + 1) * P]
                    cj, cs = k2chunk[k]
                    rh = fhi_tiles[cj][:, cs, :]
                    first = k == klo[i]
                    last = k == khi[i] - 1
                    for n0, nn in ((0, 512), (512, 256)):
                        nc.tensor.matmul(
                            ps[:, n0 : n0 + nn], lh, rh[:, n0 : n0 + nn],
                            start=first, stop=(last and n0 == 512),
                        )
                # Evacuate + store in two halves: the first half-store
                # launches while the second half is still being scaled,
                # keeping the output DMA stream dense at the tail.
                # Outputs go on the SP ring (idle after metadata) so the
                # ACT sequencer never stalls between evacuation copies.
                os = out_pool.tile([P, D], F32, name=f"os{i}", tag="os")
                for d0, dn in ((0, 384), (384, 384)):
                    nc.scalar.mul(
                        out=os[:, d0 : d0 + dn], in_=ps[:, d0 : d0 + dn],
                        mul=iv_sb[:, i : i + 1],
                    )
                    nc.sync.dma_start(
                        out=out_r[:, i, d0 : d0 + dn], in_=os[:, d0 : d0 + dn]
                    )

    nc.finalize()
    return nc


def _prepare(features, begins, ends):
    feats = np.asarray(features, dtype=np.float32)
    assert feats.shape == (B, T, D), feats.shape
    b = np.clip(np.asarray(begins).astype(np.int64), 0, T - 1)
    e = np.asarray(ends).astype(np.int64)
    # Reference gathers at most MAXWIN tokens starting at b; empty -> count 1.
    e_eff = np.clip(e, b, np.minimum(b + MAXWIN, T))
    counts = np.maximum(e_eff - b, 1).astype(np.float32)
    inv = (1.0 / counts).astype(np.float32)

    bw = b.reshape(B, NBLK, P)
    ew = e_eff.reshape(B, NBLK, P)
    klo_pc = bw.min(-1) // P  # [B, NBLK]
    khi_pc = (np.maximum(ew.max(-1) - 1, bw.min(-1)) // P) + 1
    klo = klo_pc.min(0).astype(int)
    khi = khi_pc.max(0).astype(int)
    khi = np.minimum(np.maximum(khi, klo + 1), NKT)

    # shuffle to [P, NKT, D]: partition p holds tokens {p, 128+p, ...}
    hi = np.ascontiguousarray(
        feats.astype(np.float16).reshape(B, NKT, P, D).transpose(0, 2, 1, 3)
    )

    iota = (np.arange(NKT)[None, :] * P + np.arange(P)[:, None]).astype(np.float32)
    in_maps = []
    for c in range(B):
        be = np.stack([b[c], e_eff[c]]).astype(np.int16)  # [2, W]
        metac = np.ascontiguousarray(
            np.broadcast_to(
                be.reshape(2, W // MCH, MCH).transpose(1, 0, 2)[:, None],
                (W // MCH, P, 2, MCH),
            )
        )
        ioiv = np.zeros((P, P), np.float32)
        ioiv[:, 0:NKT] = iota
        ioiv[:, NKT : NKT + NBLK] = inv[c].reshape(NBLK, P).T
        in_maps.append(
            {
                "fhi": hi[c],
                "meta": metac,
                "ioiv": ioiv,
            }
        )
    return list(klo), list(khi), in_maps


def run(features, begins, ends, trace=False):
    """Build + run on 8 NeuronCores; returns (output, BassKernelResults)."""
    klo, khi, in_maps = _prepare(features, begins, ends)
    nc = _build_program(klo, khi)
    res = run_bass_kernel_spmd(nc, in_maps, list(range(B)), trace=trace)
    out = np.stack([res.results[c]["out"] for c in range(B)], axis=0)
    return out, res


def kernel(features, begins, ends):
    out, _ = run(features, begins, ends, trace=False)
    return out


# revision 35
# speedup vs baseline: 1.0991x; 1.0991x over previous
"""Trainium2 Bass kernel for windowed mean-pooling (segment_reduce).

Computes, for each (batch b, window w):
    out[b, w, :] = mean over t in [begins[b,w], ends'[b,w]) of features[b, t, :]
where ends' = clip(ends, begins, begins + 8) (the reference gathers at most
MAX_WINDOW=8 tokens) and empty windows produce 0 (count clamped to >= 1).

Strategy (data-parallel over batch, one sample per NeuronCore):
  - The kernel is HBM-bound, so input bytes are minimized: features ship as
    fp16 (6.3 MB instead of 12.6 MB fp32; ~2e-4 rel err on the windowed
    means, and fp16 matmuls run at full PE rate unlike fp32 which lowers
    to two HW passes), begins/ends metadata as int16 broadcast rows (1 MB).
  - Slab layout in SBUF: token t on partition (t % 128), K-tile (t // 128).
  - For each 128-window output block: out_block = S^T @ F on the
    TensorEngine, where S[t, w] = (begins[w] <= t < ends[w]) is built per
    K-tile by the VectorEngine directly from the int16 rows with fused
    compare ops (S in fp16: 0/1 exact). Accumulate over the block's
    K-tiles in PSUM, scale rows by 1/count on the ScalarEngine
    (activation Copy with per-partition scale), DMA out.
  - Per-block K-tile ranges come from the host (actual index data), taking
    the union across the 8 cores so one SPMD program serves all cores
    (masks are zero outside a core's true range -> contributes nothing).
  - DMA assignment: features via GPSIMD SWDGE (descriptor generation off
    the critical sequencers, small chunks first so the PE starts early),
    metadata on SP, outputs on ACT.
"""

import os
import sys

import numpy as np

for _p in ("/opt/trn_rl_repo", "/root/.axon_site/_ro/trn_rl_repo"):
    if os.path.isdir(_p) and _p not in sys.path:
        sys.path.insert(0, _p)

from concourse import bacc, mybir  # noqa: E402
import concourse.tile as tile  # noqa: E402
from concourse.bass_utils import run_bass_kernel_spmd  # noqa: E402

B, T, D, W = 8, 4096, 768, 2048
MAXWIN = 8
P = 128
NBLK = W // P  # 16 window blocks of 128 windows
NKT = T // P  # 32 K-tiles of 128 tokens
FCHUNKS = (1, 1, 2, 4, 4, 4, 4, 4, 4, 2, 1, 1)  # K-tiles per feature DMA chunk
MCH = 512  # windows per metadata DMA chunk
F32 = mybir.dt.float32
FP16 = mybir.dt.float16
I16 = mybir.dt.int16


def _build_program(klo, khi):
    """Build the SPMD Bass program given per-block K-tile ranges [klo, khi)."""
    nc = bacc.Bacc(None)

    fhi_d = nc.declare_dram_parameter("fhi", [P, NKT, D], FP16, isOutput=False)
    meta = nc.declare_dram_parameter("meta", [W // MCH, P, 2, MCH], I16, isOutput=False)
    ioiv = nc.declare_dram_parameter("ioiv", [P, P], F32, isOutput=False)
    out_d = nc.declare_dram_parameter("out", [W, D], F32, isOutput=True)

    # token t = n*128 + p -> fhi[p, n, d] (host-shuffled for contiguous
    # per-partition DMA descriptors); window w = i*128 + p -> [p, i, d]
    fhi_r = fhi_d[:]
    out_r = out_d[:].rearrange("(n p) d -> p n d", p=P)

    # For each K-tile, the contiguous span of blocks that consume it.
    strip_rng = {}
    for k in range(NKT):
        blks = [i for i in range(NBLK) if klo[i] <= k < khi[i]]
        if blks:
            strip_rng[k] = (min(blks), max(blks) + 1)

    with tile.TileContext(nc) as tc:
        with (
            tc.tile_pool(name="metap", bufs=1) as meta_pool,
            tc.tile_pool(name="fslab", bufs=1) as f_pool,
            tc.tile_pool(name="m2p", bufs=3) as m2_pool,
            tc.tile_pool(name="maskp", bufs=6) as mask_pool,
            tc.tile_pool(name="outp", bufs=8) as out_pool,
            tc.tile_pool(name="psum", bufs=4, space="PSUM") as psum_pool,
        ):
            # iota [P, :NKT] (iota[p, k] = 128k + p), 1/count [P, NKT:NKT+NBLK],
            # zero-padded to [P, 128] so DMA descriptors stay >= 512 B.
            ioiv_sb = meta_pool.tile([P, P], F32)
            nc.sync.dma_start(out=ioiv_sb[:], in_=ioiv[:])
            io_sb = ioiv_sb[:, 0:NKT]
            iv_sb = ioiv_sb[:, NKT : NKT + NBLK]

            # begins/ends rows (pre-broadcast by the host, int16), one tile
            # per 512-window chunk so early strips only wait for their chunk.
            be_tiles = []
            for c in range(W // MCH):
                sl = slice(c * MCH, (c + 1) * MCH)
                bt = meta_pool.tile([P, 2, MCH], I16, name=f"be{c}", tag=f"be{c}")
                nc.sync.dma_start(out=bt[:], in_=meta[:][c])
                be_tiles.append(bt)

            # Feature slab chunks (fp16), small chunks first.
            fhi_tiles = []
            k2chunk = []
            k0 = 0
            for j, sz in enumerate(FCHUNKS):
                fh = f_pool.tile([P, sz, D], FP16, name=f"fh{j}", tag=f"fh{j}")
                nc.gpsimd.dma_start(out=fh[:], in_=fhi_r[:, k0 : k0 + sz, :])
                fhi_tiles.append(fh)
                for s in range(sz):
                    k2chunk.append((j, s))
                k0 += sz
            assert k0 == NKT

            # Per-K-tile mask strips over the span of blocks that use them,
            # in [token, window] layout: mask[p, w] = (b[w] <= t) * (e[w] > t)
            # with t = 128k + p.
            masks = {}
            for k in sorted(strip_rng):
                blo, bhi = strip_rng[k]
                wlo, whi = blo * P, bhi * P
                wn = whi - wlo
                m2 = m2_pool.tile([P, wn], FP16, name=f"m2_{k}", tag="m2")
                msk = mask_pool.tile([P, wn], FP16, name=f"mask_{k}", tag="mask")
                w0 = wlo
                while w0 < whi:  # split at metadata-chunk boundaries
                    w1 = min(whi, (w0 // MCH + 1) * MCH)
                    bt = be_tiles[w0 // MCH]
                    osl = slice(w0 - wlo, w1 - wlo)
                    csl = slice(w0 % MCH, w0 % MCH + (w1 - w0))
                    nc.vector.tensor_scalar(
                        m2[:, osl], bt[:, 1, csl], io_sb[:, k : k + 1], None,
                        mybir.AluOpType.is_gt,
                    )
                    nc.vector.scalar_tensor_tensor(
                        msk[:, osl], bt[:, 0, csl], io_sb[:, k : k + 1], m2[:, osl],
                        mybir.AluOpType.is_le, mybir.AluOpType.mult,
                    )
                    w0 = w1
                masks[k] = (msk, blo)

            for i in range(NBLK):
                ps = psum_pool.tile([P, D], F32, name=f"ps{i}", tag="ps")
                for k in range(klo[i], khi[i]):
                    msk, blo = masks[k]
                    lh = msk[:, (i - blo) * P : (i - blo + 1) * P]
                    cj, cs = k2chunk[k]
                    rh = fhi_tiles[cj][:, cs, :]
                    first = k == klo[i]
                    last = k == khi[i] - 1
                    for n0, nn in ((0, 512), (512, 256)):
                        nc.tensor.matmul(
                            ps[:, n0 : n0 + nn], lh, rh[:, n0 : n0 + nn],
                            start=first, stop=(last and n0 == 512),
                        )
                os = out_pool.tile([P, D], F32, name=f"os{i}", tag="os")
                nc.scalar.mul(out=os[:], in_=ps[:], mul=iv_sb[:, i : i + 1])
                # Outputs on the SP ring (idle after metadata) so the ACT
                # sequencer never stalls between evacuation copies.
                nc.sync.dma_start(out=out_r[:, i, :], in_=os[:])

    nc.finalize()
    return nc


def _prepare(features, begins, ends):
    feats = np.asarray(features, dtype=np.float32)
    assert feats.shape == (B, T, D), feats.shape
    b = np.clip(np.asarray(begins).astype(np.int64), 0, T - 1)
    e = np.asarray(ends).astype(np.int64)
    # Reference gathers at most MAXWIN tokens starting at b; empty -> count 1.
    e_eff = np.clip(e, b, np.minimum(b + MAXWIN, T))
    counts = np.maximum(e_eff - b, 1).astype(np.float32)
    inv = (1.0 / counts).astype(np.float32)

    bw = b.reshape(B, NBLK, P)
    ew = e_eff.reshape(B, NBLK, P)
    klo_pc = bw.min(-1) // P  # [B, NBLK]
    khi_pc = (np.maximum(ew.max(-1) - 1, bw.min(-1)) // P) + 1
    klo = klo_pc.min(0).astype(int)
    khi = khi_pc.max(0).astype(int)
    khi = np.minimum(np.maximum(khi, klo + 1), NKT)

    # shuffle to [P, NKT, D]: partition p holds tokens {p, 128+p, ...}
    hi = np.ascontiguousarray(
        feats.astype(np.float16).reshape(B, NKT, P, D).transpose(0, 2, 1, 3)
    )

    iota = (np.arange(NKT)[None, :] * P + np.arange(P)[:, None]).astype(np.float32)
    in_maps = []
    for c in range(B):
        be = np.stack([b[c], e_eff[c]]).astype(np.int16)  # [2, W]
        metac = np.ascontiguousarray(
            np.broadcast_to(
                be.reshape(2, W // MCH, MCH).transpose(1, 0, 2)[:, None],
                (W // MCH, P, 2, MCH),
            )
        )
        ioiv = np.zeros((P, P), np.float32)
        ioiv[:, 0:NKT] = iota
        ioiv[:, NKT : NKT + NBLK] = inv[c].reshape(NBLK, P).T
        in_maps.append(
            {
                "fhi": hi[c],
                "meta": metac,
                "ioiv": ioiv,
            }
        )
    return list(klo), list(khi), in_maps


def run(features, begins, ends, trace=False):
    """Build + run on 8 NeuronCores; returns (output, BassKernelResults)."""
    klo, khi, in_maps = _prepare(features, begins, ends)
    nc = _build_program(klo, khi)
    res = run_bass_kernel_spmd(nc, in_maps, list(range(B)), trace=trace)
    out = np.stack([res.results[c]["out"] for c in range(B)], axis=0)
    return out, res


def kernel(features, begins, ends):
    out, _ = run(features, begins, ends, trace=False)
    return out


# revision 36
# speedup vs baseline: 1.1347x; 1.0324x over previous
"""Trainium2 Bass kernel for windowed mean-pooling (segment_reduce).

Computes, for each (batch b, window w):
    out[b, w, :] = mean over t in [begins[b,w], ends'[b,w]) of features[b, t, :]
where ends' = clip(ends, begins, begins + 8) (the reference gathers at most
MAX_WINDOW=8 tokens) and empty windows produce 0 (count clamped to >= 1).

Strategy (data-parallel over batch, one sample per NeuronCore):
  - The kernel is HBM-bound, so input bytes are minimized: features ship as
    fp16 (6.3 MB instead of 12.6 MB fp32; ~2e-4 rel err on the windowed
    means, and fp16 matmuls run at full PE rate unlike fp32 which lowers
    to two HW passes), begins/ends metadata as int16 broadcast rows (1 MB).
  - Slab layout in SBUF: token t on partition (t % 128), K-tile (t // 128).
  - For each 128-window output block: out_block = S^T @ F on the
    TensorEngine, where S[t, w] = (begins[w] <= t < ends[w]) is built per
    K-tile by the VectorEngine directly from the int16 rows with fused
    compare ops (S in fp16: 0/1 exact). Accumulate over the block's
    K-tiles in PSUM, scale rows by 1/count on the ScalarEngine
    (activation Copy with per-partition scale), DMA out.
  - Per-block K-tile ranges come from the host (actual index data), taking
    the union across the 8 cores so one SPMD program serves all cores
    (masks are zero outside a core's true range -> contributes nothing).
  - DMA assignment: features via GPSIMD SWDGE (descriptor generation off
    the critical sequencers, small chunks first so the PE starts early),
    metadata on SP, outputs on ACT.
"""

import os
import sys

import numpy as np

for _p in ("/opt/trn_rl_repo", "/root/.axon_site/_ro/trn_rl_repo"):
    if os.path.isdir(_p) and _p not in sys.path:
        sys.path.insert(0, _p)

from concourse import bacc, mybir  # noqa: E402
import concourse.tile as tile  # noqa: E402
from concourse.bass_utils import run_bass_kernel_spmd  # noqa: E402

B, T, D, W = 8, 4096, 768, 2048
MAXWIN = 8
P = 128
NBLK = W // P  # 16 window blocks of 128 windows
NKT = T // P  # 32 K-tiles of 128 tokens
FCHUNKS = (1, 1, 2, 4, 4, 4, 4, 4, 4, 2, 1, 1)  # K-tiles per feature DMA chunk
MCH = 512  # windows per metadata DMA chunk
F32 = mybir.dt.float32
FP16 = mybir.dt.float16
I16 = mybir.dt.int16


def _build_program(klo, khi):
    """Build the SPMD Bass program given per-block K-tile ranges [klo, khi)."""
    nc = bacc.Bacc(None)

    fhi_d = nc.declare_dram_parameter("fhi", [P, NKT, D], FP16, isOutput=False)
    meta = nc.declare_dram_parameter("meta", [1, 2, W], FP16, isOutput=False)
    ioiv = nc.declare_dram_parameter("ioiv", [P, P], F32, isOutput=False)
    out_d = nc.declare_dram_parameter("out", [W, D], F32, isOutput=True)

    # token t = n*128 + p -> fhi[p, n, d] (host-shuffled for contiguous
    # per-partition DMA descriptors); window w = i*128 + p -> [p, i, d]
    fhi_r = fhi_d[:]
    out_r = out_d[:].rearrange("(n p) d -> p n d", p=P)

    # For each K-tile, the contiguous span of blocks that consume it.
    strip_rng = {}
    for k in range(NKT):
        blks = [i for i in range(NBLK) if klo[i] <= k < khi[i]]
        if blks:
            strip_rng[k] = (min(blks), max(blks) + 1)

    with tile.TileContext(nc) as tc:
        with (
            tc.tile_pool(name="metap", bufs=1) as meta_pool,
            tc.tile_pool(name="fslab", bufs=1) as f_pool,
            tc.tile_pool(name="m2p", bufs=3) as m2_pool,
            tc.tile_pool(name="maskp", bufs=6) as mask_pool,
            tc.tile_pool(name="outp", bufs=8) as out_pool,
            tc.tile_pool(name="psum", bufs=3, space="PSUM") as psum_pool,
            tc.tile_pool(name="psumb", bufs=2, space="PSUM") as psumb_pool,
        ):
            # iota [P, :NKT] (iota[p, k] = 128k + p), 1/count [P, NKT:NKT+NBLK],
            # zero-padded to [P, 128] so DMA descriptors stay >= 512 B.
            ioiv_sb = meta_pool.tile([P, P], F32)
            nc.sync.dma_start(out=ioiv_sb[:], in_=ioiv[:])
            io_sb = ioiv_sb[:, 0:NKT]
            iv_sb = ioiv_sb[:, NKT : NKT + NBLK]

            # begins/ends arrive as ONE 8 KB fp16 row (values shifted by
            # -2048 so 0..4096 are all fp16-exact) and are broadcast across
            # the 128 partitions with K=1 fp16 ones-matmuls on the idle PE,
            # saving ~1 MB of HBM traffic.
            rows_sb = meta_pool.tile([1, 2, W], FP16)
            nc.sync.dma_start(out=rows_sb[:], in_=meta[:])
            ones_sb = meta_pool.tile([1, P], FP16)
            nc.vector.memset(ones_sb[:], 1.0)
            be_sb = meta_pool.tile([P, 2, W], FP16)
            for h in range(2):
                for s in range(W // MCH):
                    sl = slice(s * MCH, (s + 1) * MCH)
                    pb = psumb_pool.tile([P, MCH], F32, name=f"pb{h}_{s}", tag="pb")
                    nc.tensor.matmul(
                        pb[:], ones_sb[:], rows_sb[:, h, sl], start=True, stop=True
                    )
                    nc.vector.tensor_copy(out=be_sb[:, h, sl], in_=pb[:])

            # Feature slab chunks (fp16), small chunks first.
            fhi_tiles = []
            k2chunk = []
            k0 = 0
            for j, sz in enumerate(FCHUNKS):
                fh = f_pool.tile([P, sz, D], FP16, name=f"fh{j}", tag=f"fh{j}")
                nc.gpsimd.dma_start(out=fh[:], in_=fhi_r[:, k0 : k0 + sz, :])
                fhi_tiles.append(fh)
                for s in range(sz):
                    k2chunk.append((j, s))
                k0 += sz
            assert k0 == NKT

            # Per-K-tile mask strips over the span of blocks that use them,
            # in [token, window] layout: mask[p, w] = (b[w] <= t) * (e[w] > t)
            # with t = 128k + p.
            masks = {}
            for k in sorted(strip_rng):
                blo, bhi = strip_rng[k]
                wlo, whi = blo * P, bhi * P
                wn = whi - wlo
                m2 = m2_pool.tile([P, wn], FP16, name=f"m2_{k}", tag="m2")
                msk = mask_pool.tile([P, wn], FP16, name=f"mask_{k}", tag="mask")
                nc.vector.tensor_scalar(
                    m2[:], be_sb[:, 1, wlo:whi], io_sb[:, k : k + 1], None,
                    mybir.AluOpType.is_gt,
                )
                nc.vector.scalar_tensor_tensor(
                    msk[:], be_sb[:, 0, wlo:whi], io_sb[:, k : k + 1], m2[:],
                    mybir.AluOpType.is_le, mybir.AluOpType.mult,
                )
                masks[k] = (msk, blo)

            for i in range(NBLK):
                ps = psum_pool.tile([P, D], F32, name=f"ps{i}", tag="ps")
                for k in range(klo[i], khi[i]):
                    msk, blo = masks[k]
                    lh = msk[:, (i - blo) * P : (i - blo + 1) * P]
                    cj, cs = k2chunk[k]
                    rh = fhi_tiles[cj][:, cs, :]
                    first = k == klo[i]
                    last = k == khi[i] - 1
                    for n0, nn in ((0, 512), (512, 256)):
                        nc.tensor.matmul(
                            ps[:, n0 : n0 + nn], lh, rh[:, n0 : n0 + nn],
                            start=first, stop=(last and n0 == 512),
                        )
                os = out_pool.tile([P, D], F32, name=f"os{i}", tag="os")
                nc.scalar.mul(out=os[:], in_=ps[:], mul=iv_sb[:, i : i + 1])
                # Outputs on the SP ring (idle after metadata) so the ACT
                # sequencer never stalls between evacuation copies.
                nc.sync.dma_start(out=out_r[:, i, :], in_=os[:])

    nc.finalize()
    return nc


def _prepare(features, begins, ends):
    feats = np.asarray(features, dtype=np.float32)
    assert feats.shape == (B, T, D), feats.shape
    b = np.clip(np.asarray(begins).astype(np.int64), 0, T - 1)
    e = np.asarray(ends).astype(np.int64)
    # Reference gathers at most MAXWIN tokens starting at b; empty -> count 1.
    e_eff = np.clip(e, b, np.minimum(b + MAXWIN, T))
    counts = np.maximum(e_eff - b, 1).astype(np.float32)
    inv = (1.0 / counts).astype(np.float32)

    bw = b.reshape(B, NBLK, P)
    ew = e_eff.reshape(B, NBLK, P)
    klo_pc = bw.min(-1) // P  # [B, NBLK]
    khi_pc = (np.maximum(ew.max(-1) - 1, bw.min(-1)) // P) + 1
    klo = klo_pc.min(0).astype(int)
    khi = khi_pc.max(0).astype(int)
    khi = np.minimum(np.maximum(khi, klo + 1), NKT)

    # shuffle to [P, NKT, D]: partition p holds tokens {p, 128+p, ...}
    hi = np.ascontiguousarray(
        feats.astype(np.float16).reshape(B, NKT, P, D).transpose(0, 2, 1, 3)
    )

    iota = (
        np.arange(NKT)[None, :] * P + np.arange(P)[:, None] - 2048
    ).astype(np.float32)
    in_maps = []
    for c in range(B):
        metac = np.ascontiguousarray(
            (np.stack([b[c], e_eff[c]]) - 2048).astype(np.float16).reshape(1, 2, W)
        )
        ioiv = np.zeros((P, P), np.float32)
        ioiv[:, 0:NKT] = iota
        ioiv[:, NKT : NKT + NBLK] = inv[c].reshape(NBLK, P).T
        in_maps.append(
            {
                "fhi": hi[c],
                "meta": metac,
                "ioiv": ioiv,
            }
        )
    return list(klo), list(khi), in_maps


def run(features, begins, ends, trace=False):
    """Build + run on 8 NeuronCores; returns (output, BassKernelResults)."""
    klo, khi, in_maps = _prepare(features, begins, ends)
    nc = _build_program(klo, khi)
    res = run_bass_kernel_spmd(nc, in_maps, list(range(B)), trace=trace)
    out = np.stack([res.results[c]["out"] for c in range(B)], axis=0)
    return out, res


def kernel(features, begins, ends):
    out, _ = run(features, begins, ends, trace=False)
    return out


# revision 37
# speedup vs baseline: 1.1634x; 1.0253x over previous
"""Trainium2 Bass kernel for windowed mean-pooling (segment_reduce).

Computes, for each (batch b, window w):
    out[b, w, :] = mean over t in [begins[b,w], ends'[b,w]) of features[b, t, :]
where ends' = clip(ends, begins, begins + 8) (the reference gathers at most
MAX_WINDOW=8 tokens) and empty windows produce 0 (count clamped to >= 1).

Strategy (data-parallel over batch, one sample per NeuronCore):
  - The kernel is HBM-bound, so input bytes are minimized: features ship as
    fp16 (6.3 MB instead of 12.6 MB fp32; ~2e-4 rel err on the windowed
    means, and fp16 matmuls run at full PE rate unlike fp32 which lowers
    to two HW passes); begins/ends arrive as one 8 KB fp16 row (values
    shifted by -2048 so 0..4096 are fp16-exact) and are broadcast across
    partitions on-chip with K=1 fp16 ones-matmuls on the idle PE.
  - Slab layout in SBUF: token t on partition (t % 128), K-tile (t // 128).
  - For each 128-window output block: out_block = S^T @ F on the
    TensorEngine, where S[t, w] = (begins[w] <= t < ends[w]) is built per
    K-tile by the VectorEngine from the broadcast rows with fused
    compare ops (S in fp16: 0/1 exact). Accumulate over the block's
    K-tiles in PSUM, scale rows by 1/count on the ScalarEngine
    (activation Copy with per-partition scale), DMA out.
  - Per-block K-tile ranges come from the host (actual index data), taking
    the union across the 8 cores so one SPMD program serves all cores
    (masks are zero outside a core's true range -> contributes nothing).
  - DMA assignment: features via GPSIMD SWDGE (descriptor generation off
    the critical sequencers, small chunks first so the PE starts early),
    metadata on SP, outputs on ACT.
"""

import os
import sys

import numpy as np

for _p in ("/opt/trn_rl_repo", "/root/.axon_site/_ro/trn_rl_repo"):
    if os.path.isdir(_p) and _p not in sys.path:
        sys.path.insert(0, _p)

from concourse import bacc, mybir  # noqa: E402
import concourse.tile as tile  # noqa: E402
from concourse.bass_utils import run_bass_kernel_spmd  # noqa: E402

B, T, D, W = 8, 4096, 768, 2048
MAXWIN = 8
P = 128
NBLK = W // P  # 16 window blocks of 128 windows
NKT = T // P  # 32 K-tiles of 128 tokens
FCHUNKS = (1, 1, 2, 4, 4, 4, 4, 4, 4, 2, 1, 1)  # K-tiles per feature DMA chunk
MCH = 512  # windows per metadata DMA chunk
F32 = mybir.dt.float32
FP16 = mybir.dt.float16
I16 = mybir.dt.int16


def _build_program(klo, khi):
    """Build the SPMD Bass program given per-block K-tile ranges [klo, khi)."""
    nc = bacc.Bacc(None)

    fhi_d = nc.declare_dram_parameter("fhi", [P, NKT, D], FP16, isOutput=False)
    meta = nc.declare_dram_parameter("meta", [1, 2, W], FP16, isOutput=False)
    ioiv = nc.declare_dram_parameter("ioiv", [P, P], F32, isOutput=False)
    out_d = nc.declare_dram_parameter("out", [W, D], F32, isOutput=True)

    # token t = n*128 + p -> fhi[p, n, d] (host-shuffled for contiguous
    # per-partition DMA descriptors); window w = i*128 + p -> [p, i, d]
    fhi_r = fhi_d[:]
    out_r = out_d[:].rearrange("(n p) d -> p n d", p=P)

    # For each K-tile, the contiguous span of blocks that consume it.
    strip_rng = {}
    for k in range(NKT):
        blks = [i for i in range(NBLK) if klo[i] <= k < khi[i]]
        if blks:
            strip_rng[k] = (min(blks), max(blks) + 1)

    with tile.TileContext(nc) as tc:
        with (
            tc.tile_pool(name="metap", bufs=1) as meta_pool,
            tc.tile_pool(name="fslab", bufs=1) as f_pool,
            tc.tile_pool(name="m2p", bufs=3) as m2_pool,
            tc.tile_pool(name="maskp", bufs=6) as mask_pool,
            tc.tile_pool(name="outp", bufs=8) as out_pool,
            tc.tile_pool(name="psum", bufs=3, space="PSUM") as psum_pool,
            tc.tile_pool(name="psumb", bufs=2, space="PSUM") as psumb_pool,
        ):
            # iota [P, :NKT] (iota[p, k] = 128k + p), 1/count [P, NKT:NKT+NBLK],
            # zero-padded to [P, 128] so DMA descriptors stay >= 512 B.
            ioiv_sb = meta_pool.tile([P, P], F32)
            nc.sync.dma_start(out=ioiv_sb[:], in_=ioiv[:])
            io_sb = ioiv_sb[:, 0:NKT]
            iv_sb = ioiv_sb[:, NKT : NKT + NBLK]

            # begins/ends arrive as ONE 8 KB fp16 row (values shifted by
            # -2048 so 0..4096 are all fp16-exact) and are broadcast across
            # the 128 partitions with K=1 fp16 ones-matmuls on the idle PE,
            # saving ~1 MB of HBM traffic.
            rows_sb = meta_pool.tile([1, 2, W], FP16)
            nc.sync.dma_start(out=rows_sb[:], in_=meta[:])
            ones_sb = meta_pool.tile([1, P], FP16)
            nc.vector.memset(ones_sb[:], 1.0)
            be_sb = meta_pool.tile([P, 2, W], FP16)
            for h in range(2):
                for s in range(W // MCH):
                    sl = slice(s * MCH, (s + 1) * MCH)
                    pb = psumb_pool.tile([P, MCH], F32, name=f"pb{h}_{s}", tag="pb")
                    nc.tensor.matmul(
                        pb[:], ones_sb[:], rows_sb[:, h, sl], start=True, stop=True
                    )
                    nc.vector.tensor_copy(out=be_sb[:, h, sl], in_=pb[:])

            # Feature slab chunks (fp16), small chunks first.
            fhi_tiles = []
            k2chunk = []
            k0 = 0
            for j, sz in enumerate(FCHUNKS):
                fh = f_pool.tile([P, sz, D], FP16, name=f"fh{j}", tag=f"fh{j}")
                nc.gpsimd.dma_start(out=fh[:], in_=fhi_r[:, k0 : k0 + sz, :])
                fhi_tiles.append(fh)
                for s in range(sz):
                    k2chunk.append((j, s))
                k0 += sz
            assert k0 == NKT

            # Per-K-tile mask strips over the span of blocks that use them,
            # in [token, window] layout: mask[p, w] = (b[w] <= t) * (e[w] > t)
            # with t = 128k + p.
            masks = {}
            for k in sorted(strip_rng):
                blo, bhi = strip_rng[k]
                wlo, whi = blo * P, bhi * P
                wn = whi - wlo
                m2 = m2_pool.tile([P, wn], FP16, name=f"m2_{k}", tag="m2")
                msk = mask_pool.tile([P, wn], FP16, name=f"mask_{k}", tag="mask")
                nc.vector.tensor_scalar(
                    m2[:], be_sb[:, 1, wlo:whi], io_sb[:, k : k + 1], None,
                    mybir.AluOpType.is_gt,
                )
                nc.vector.scalar_tensor_tensor(
                    msk[:], be_sb[:, 0, wlo:whi], io_sb[:, k : k + 1], m2[:],
                    mybir.AluOpType.is_le, mybir.AluOpType.mult,
                )
                masks[k] = (msk, blo)

            for i in range(NBLK):
                ps = psum_pool.tile([P, D], F32, name=f"ps{i}", tag="ps")
                for k in range(klo[i], khi[i]):
                    msk, blo = masks[k]
                    lh = msk[:, (i - blo) * P : (i - blo + 1) * P]
                    cj, cs = k2chunk[k]
                    rh = fhi_tiles[cj][:, cs, :]
                    first = k == klo[i]
                    last = k == khi[i] - 1
                    for n0, nn in ((0, 512), (512, 256)):
                        nc.tensor.matmul(
                            ps[:, n0 : n0 + nn], lh, rh[:, n0 : n0 + nn],
                            start=first, stop=(last and n0 == 512),
                        )
                os = out_pool.tile([P, D], F32, name=f"os{i}", tag="os")
                nc.scalar.mul(out=os[:], in_=ps[:], mul=iv_sb[:, i : i + 1])
                # Outputs on the SP ring (idle after metadata) so the ACT
                # sequencer never stalls between evacuation copies.
                nc.sync.dma_start(out=out_r[:, i, :], in_=os[:])

    nc.finalize()
    return nc


def _prepare(features, begins, ends):
    feats = np.asarray(features, dtype=np.float32)
    assert feats.shape == (B, T, D), feats.shape
    b = np.clip(np.asarray(begins).astype(np.int64), 0, T - 1)
    e = np.asarray(ends).astype(np.int64)
    # Reference gathers at most MAXWIN tokens starting at b; empty -> count 1.
    e_eff = np.clip(e, b, np.minimum(b + MAXWIN, T))
    counts = np.maximum(e_eff - b, 1).astype(np.float32)
    inv = (1.0 / counts).astype(np.float32)

    bw = b.reshape(B, NBLK, P)
    ew = e_eff.reshape(B, NBLK, P)
    klo_pc = bw.min(-1) // P  # [B, NBLK]
    khi_pc = (np.maximum(ew.max(-1) - 1, bw.min(-1)) // P) + 1
    klo = klo_pc.min(0).astype(int)
    khi = khi_pc.max(0).astype(int)
    khi = np.minimum(np.maximum(khi, klo + 1), NKT)

    # shuffle to [P, NKT, D]: partition p holds tokens {p, 128+p, ...}
    hi = np.ascontiguousarray(
        feats.astype(np.float16).reshape(B, NKT, P, D).transpose(0, 2, 1, 3)
    )

    iota = (
        np.arange(NKT)[None, :] * P + np.arange(P)[:, None] - 2048
    ).astype(np.float32)
    in_maps = []
    for c in range(B):
        metac = np.ascontiguousarray(
            (np.stack([b[c], e_eff[c]]) - 2048).astype(np.float16).reshape(1, 2, W)
        )
        ioiv = np.zeros((P, P), np.float32)
        ioiv[:, 0:NKT] = iota
        ioiv[:, NKT : NKT + NBLK] = inv[c].reshape(NBLK, P).T
        in_maps.append(
            {
                "fhi": hi[c],
                "meta": metac,
                "ioiv": ioiv,
            }
        )
    return list(klo), list(khi), in_maps


def run(features, begins, ends, trace=False):
    """Build + run on 8 NeuronCores; returns (output, BassKernelResults)."""
    klo, khi, in_maps = _prepare(features, begins, ends)
    nc = _build_program(klo, khi)
    res = run_bass_kernel_spmd(nc, in_maps, list(range(B)), trace=trace)
    out = np.stack([res.results[c]["out"] for c in range(B)], axis=0)
    return out, res


def kernel(features, begins, ends):
    out, _ = run(features, begins, ends, trace=False)
    return out


# revision 38
# speedup vs baseline: 1.1764x; 1.0111x over previous
"""Trainium2 Bass kernel for windowed mean-pooling (segment_reduce).

Computes, for each (batch b, window w):
    out[b, w, :] = mean over t in [begins[b,w], ends'[b,w]) of features[b, t, :]
where ends' = clip(ends, begins, begins + 8) (the reference gathers at most
MAX_WINDOW=8 tokens) and empty windows produce 0 (count clamped to >= 1).

Strategy (data-parallel over batch, one sample per NeuronCore):
  - The kernel is HBM-bound, so input bytes are minimized: features ship as
    fp16 (6.3 MB instead of 12.6 MB fp32; ~2e-4 rel err on the windowed
    means, and fp16 matmuls run at full PE rate unlike fp32 which lowers
    to two HW passes); begins/ends arrive as one 8 KB fp16 row (values
    shifted by -2048 so 0..4096 are fp16-exact) and are broadcast across
    partitions on-chip with K=1 fp16 ones-matmuls on the idle PE.
  - Slab layout in SBUF: token t on partition (t % 128), K-tile (t // 128).
  - For each 128-window output block: out_block = S^T @ F on the
    TensorEngine, where S[t, w] = (begins[w] <= t < ends[w]) is built per
    K-tile by the VectorEngine from the broadcast rows with fused
    compare ops (S in fp16: 0/1 exact). Accumulate over the block's
    K-tiles in PSUM, scale rows by 1/count on the ScalarEngine
    (activation Copy with per-partition scale), DMA out.
  - Per-block K-tile ranges come from the host (actual index data), taking
    the union across the 8 cores so one SPMD program serves all cores
    (masks are zero outside a core's true range -> contributes nothing).
  - DMA assignment: features via GPSIMD SWDGE (descriptor generation off
    the critical sequencers, small chunks first so the PE starts early),
    metadata on SP, outputs on ACT.
"""

import os
import sys

import numpy as np

for _p in ("/opt/trn_rl_repo", "/root/.axon_site/_ro/trn_rl_repo"):
    if os.path.isdir(_p) and _p not in sys.path:
        sys.path.insert(0, _p)

from concourse import bacc, mybir  # noqa: E402
import concourse.tile as tile  # noqa: E402
from concourse.bass_utils import run_bass_kernel_spmd  # noqa: E402

B, T, D, W = 8, 4096, 768, 2048
MAXWIN = 8
P = 128
NBLK = W // P  # 16 window blocks of 128 windows
NKT = T // P  # 32 K-tiles of 128 tokens
FCHUNKS = (1, 1, 2, 4, 4, 4, 4, 4, 4, 2, 1, 1)  # K-tiles per feature DMA chunk
MCH = 512  # windows per metadata DMA chunk
F32 = mybir.dt.float32
FP16 = mybir.dt.float16
I16 = mybir.dt.int16


def _build_program(klo, khi):
    """Build the SPMD Bass program given per-block K-tile ranges [klo, khi)."""
    nc = bacc.Bacc(None)

    fhi_d = nc.declare_dram_parameter("fhi", [P, NKT, D], FP16, isOutput=False)
    meta = nc.declare_dram_parameter("meta", [1, 2, W], FP16, isOutput=False)
    ioiv = nc.declare_dram_parameter("ioiv", [P, P], F32, isOutput=False)
    out_d = nc.declare_dram_parameter("out", [W, D], F32, isOutput=True)

    # token t = n*128 + p -> fhi[p, n, d] (host-shuffled for contiguous
    # per-partition DMA descriptors); window w = i*128 + p -> [p, i, d]
    fhi_r = fhi_d[:]
    out_r = out_d[:].rearrange("(n p) d -> p n d", p=P)

    # For each K-tile, the contiguous span of blocks that consume it.
    strip_rng = {}
    for k in range(NKT):
        blks = [i for i in range(NBLK) if klo[i] <= k < khi[i]]
        if blks:
            strip_rng[k] = (min(blks), max(blks) + 1)

    with tile.TileContext(nc) as tc:
        with (
            tc.tile_pool(name="metap", bufs=1) as meta_pool,
            tc.tile_pool(name="fslab", bufs=1) as f_pool,
            tc.tile_pool(name="m2p", bufs=3) as m2_pool,
            tc.tile_pool(name="maskp", bufs=6) as mask_pool,
            tc.tile_pool(name="outp", bufs=8) as out_pool,
            tc.tile_pool(name="psum", bufs=4, space="PSUM") as psum_pool,
        ):
            # iota [P, :NKT] (iota[p, k] = 128k + p), 1/count [P, NKT:NKT+NBLK],
            # zero-padded to [P, 128] so DMA descriptors stay >= 512 B.
            ioiv_sb = meta_pool.tile([P, P], F32)
            nc.sync.dma_start(out=ioiv_sb[:], in_=ioiv[:])
            io_sb = ioiv_sb[:, 0:NKT]
            iv_sb = ioiv_sb[:, NKT : NKT + NBLK]

            # begins/ends arrive as ONE 8 KB fp16 row (values shifted by
            # -2048 so 0..4096 are all fp16-exact) and are broadcast across
            # the 128 partitions with K=1 fp16 ones-matmuls on the idle PE,
            # saving ~1 MB of HBM traffic.
            rows_sb = meta_pool.tile([1, 2, W], FP16)
            nc.sync.dma_start(out=rows_sb[:], in_=meta[:])
            ones_sb = meta_pool.tile([1, P], FP16)
            nc.vector.memset(ones_sb[:], 1.0)
            be_sb = meta_pool.tile([P, 2, W], FP16)
            for h in range(2):
                for s in range(W // MCH):
                    sl = slice(s * MCH, (s + 1) * MCH)
                    pb = psum_pool.tile([P, MCH], F32, name=f"pb{h}_{s}", tag="ps")
                    nc.tensor.matmul(
                        pb[:], ones_sb[:], rows_sb[:, h, sl], start=True, stop=True
                    )
                    nc.vector.tensor_copy(out=be_sb[:, h, sl], in_=pb[:])

            # Feature slab chunks (fp16), small chunks first.
            fhi_tiles = []
            k2chunk = []
            k0 = 0
            for j, sz in enumerate(FCHUNKS):
                fh = f_pool.tile([P, sz, D], FP16, name=f"fh{j}", tag=f"fh{j}")
                nc.gpsimd.dma_start(out=fh[:], in_=fhi_r[:, k0 : k0 + sz, :])
                fhi_tiles.append(fh)
                for s in range(sz):
                    k2chunk.append((j, s))
                k0 += sz
            assert k0 == NKT

            # Per-K-tile mask strips over the span of blocks that use them,
            # in [token, window] layout: mask[p, w] = (b[w] <= t) * (e[w] > t)
            # with t = 128k + p.
            masks = {}
            for k in sorted(strip_rng):
                blo, bhi = strip_rng[k]
                wlo, whi = blo * P, bhi * P
                wn = whi - wlo
                m2 = m2_pool.tile([P, wn], FP16, name=f"m2_{k}", tag="m2")
                msk = mask_pool.tile([P, wn], FP16, name=f"mask_{k}", tag="mask")
                nc.vector.tensor_scalar(
                    m2[:], be_sb[:, 1, wlo:whi], io_sb[:, k : k + 1], None,
                    mybir.AluOpType.is_gt,
                )
                nc.vector.scalar_tensor_tensor(
                    msk[:], be_sb[:, 0, wlo:whi], io_sb[:, k : k + 1], m2[:],
                    mybir.AluOpType.is_le, mybir.AluOpType.mult,
                )
                masks[k] = (msk, blo)

            for i in range(NBLK):
                ps = psum_pool.tile([P, D], F32, name=f"ps{i}", tag="ps")
                for k in range(klo[i], khi[i]):
                    msk, blo = masks[k]
                    lh = msk[:, (i - blo) * P : (i - blo + 1) * P]
                    cj, cs = k2chunk[k]
                    rh = fhi_tiles[cj][:, cs, :]
                    first = k == klo[i]
                    last = k == khi[i] - 1
                    for n0, nn in ((0, 512), (512, 256)):
                        nc.tensor.matmul(
                            ps[:, n0 : n0 + nn], lh, rh[:, n0 : n0 + nn],
                            start=first, stop=(last and n0 == 512),
                        )
                os = out_pool.tile([P, D], F32, name=f"os{i}", tag="os")
                nc.scalar.mul(out=os[:], in_=ps[:], mul=iv_sb[:, i : i + 1])
                # Outputs on the SP ring (idle after metadata) so the ACT
                # sequencer never stalls between evacuation copies.
                nc.sync.dma_start(out=out_r[:, i, :], in_=os[:])

    nc.finalize()
    return nc


def _prepare(features, begins, ends):
    feats = np.asarray(features, dtype=np.float32)
    assert feats.shape == (B, T, D), feats.shape
    b = np.clip(np.asarray(begins).astype(np.int64), 0, T - 1)
    e = np.asarray(ends).astype(np.int64)
    # Reference gathers at most MAXWIN tokens starting at b; empty -> count 1.
    e_eff = np.clip(e, b, np.minimum(b + MAXWIN, T))
    counts = np.maximum(e_eff - b, 1).astype(np.float32)
    inv = (1.0 / counts).astype(np.float32)

    bw = b.reshape(B, NBLK, P)
    ew = e_eff.reshape(B, NBLK, P)
    klo_pc = bw.min(-1) // P  # [B, NBLK]
    khi_pc = (np.maximum(ew.max(-1) - 1, bw.min(-1)) // P) + 1
    klo = klo_pc.min(0).astype(int)
    khi = khi_pc.max(0).astype(int)
    khi = np.minimum(np.maximum(khi, klo + 1), NKT)

    # shuffle to [P, NKT, D]: partition p holds tokens {p, 128+p, ...}
    hi = np.ascontiguousarray(
        feats.astype(np.float16).reshape(B, NKT, P, D).transpose(0, 2, 1, 3)
    )

    iota = (
        np.arange(NKT)[None, :] * P + np.arange(P)[:, None] - 2048
    ).astype(np.float32)
    in_maps = []
    for c in range(B):
        metac = np.ascontiguousarray(
            (np.stack([b[c], e_eff[c]]) - 2048).astype(np.float16).reshape(1, 2, W)
        )
        ioiv = np.zeros((P, P), np.float32)
        ioiv[:, 0:NKT] = iota
        ioiv[:, NKT : NKT + NBLK] = inv[c].reshape(NBLK, P).T
        in_maps.append(
            {
                "fhi": hi[c],
                "meta": metac,
                "ioiv": ioiv,
            }
        )
    return list(klo), list(khi), in_maps


def run(features, begins, ends, trace=False):
    """Build + run on 8 NeuronCores; returns (output, BassKernelResults)."""
    klo, khi, in_maps = _prepare(features, begins, ends)
    nc = _build_program(klo, khi)
    res = run_bass_kernel_spmd(nc, in_maps, list(range(B)), trace=trace)
    out = np.stack([res.results[c]["out"] for c in range(B)], axis=0)
    return out, res


def kernel(features, begins, ends):
    out, _ = run(features, begins, ends, trace=False)
    return out
